# revision 12
# baseline (speedup 1.0000x reference)
"""Differential Transformer Encoder — Trainium2 Bass kernel, 8-core SPMD.

Sharding: 8 slabs = (batch b in 0..3, sequence half in 0..1), each core owns
[512 tokens, D=1024] of one batch element. All ops are token-parallel with
replicated weights except attention, which exchanges K/V halves between the
two cores of a pair via AllGather.

Math fold: s2 = q @ roll(k,1)^T means softmax(s2) = roll(softmax(s1)) along
keys (z2 == z1 with the all-ones mask), so
    o = lam * (softmax(s1) - softmax(s2)) @ v = lam * softmax(s1) @ v'
with v'[j] = v[j] - v[(j+1) mod L].  The v-bias cancels exactly in v'.

On-chip layout: "feature-major" [F, R] tensors stored as SBUF [128, (F/128)*R],
tile kt at cols [kt*R, (kt+1)*R).  LayerNorm reductions (over features =
partitions) run as ones-vector matmuls on the PE; per-token stats are
broadcast across partitions with gpsimd.partition_broadcast.
"""

import numpy as np
import ml_dtypes

import concourse.bass as bass
import concourse.mybir as mybir
from concourse import bacc
from concourse.tile import TileContext
from concourse.bass_utils import run_bass_kernel_spmd

BF16 = mybir.dt.bfloat16
F32 = mybir.dt.float32
AF = mybir.ActivationFunctionType
OP = mybir.AluOpType

N_CORES = 8
B, L, D, H, DFF, LAYERS = 4, 1024, 1024, 16, 4096, 2
HD = D // H
R = 512            # tokens per core
NK = D // 128      # 8 feature k-tiles
NKF = DFF // 128   # 32
SCALE = float(D) ** -0.5

CC_K = 128 * NK * R          # 524288 elems: k_T local half
CC_V = R * D                 # 524288 elems: v rows local half
CC_N = CC_K + CC_V

# ---------------------------------------------------------------------------
# parameter-column packing (shared between device build and host prep)

def _par_spec():
    spec = [
        ("cw0", 8), ("cw1", 8), ("cw2", 8), ("cb", 8),
        ("dlng", 8), ("dlnb", 8), ("dgb", 8),
        ("cglng", 16), ("cglnb", 16), ("cgb1", 8),
        ("preg", 8), ("preb", 8),
    ]
    for l in range(LAYERS):
        spec += [
            (f"qkb{l}", 16), (f"outb{l}", 8),
            (f"n1g{l}", 8), (f"n1b{l}", 8),
            (f"fng{l}", 8), (f"fnb{l}", 8),
            (f"fb1{l}", 32), (f"fb2{l}", 8),
            (f"n2g{l}", 8), (f"n2b{l}", 8),
        ]
    spec += [("alpha", 1), ("cgb2", 1), ("lam0", 1), ("lam1", 1), ("eps5", 1), ("eps6", 1)]
    off, out = 0, {}
    for name, n in spec:
        out[name] = off
        off += n
    return out, off

PAR_OFF, PAR_COLS = _par_spec()

DEBUG_TAPS = []   # list of (name, n_cols_f32) filled at build when debugging


# ---------------------------------------------------------------------------
# device kernel build

def _build():
    nc = bacc.Bacc(num_devices=N_CORES)

    x_d = nc.declare_dram_parameter("x", [128, NK * (R + 2)], BF16, isOutput=False)
    cs_d = nc.declare_dram_parameter("cs", [128, 2 * NK * R], BF16, isOutput=False)
    spm_d = nc.declare_dram_parameter("spm", [128, 128], BF16, isOutput=False)
    par_d = nc.declare_dram_parameter("par", [128, PAR_COLS], F32, isOutput=False)
    cgw2_d = nc.declare_dram_parameter("cgw2", [128, NK], BF16, isOutput=False)
    dgw_d = nc.declare_dram_parameter("dgw", [128, NK * D], BF16, isOutput=False)
    cgw1_d = nc.declare_dram_parameter("cgw1", [128, NK * 2 * D], BF16, isOutput=False)
    qkw_d, wv_d, ow_d, fw1_d, fw2_d = [], [], [], [], []
    for l in range(LAYERS):
        qkw_d.append(nc.declare_dram_parameter(f"qkw{l}", [128, 16 * D], BF16, isOutput=False))
        wv_d.append(nc.declare_dram_parameter(f"wv{l}", [128, NK * D], BF16, isOutput=False))
        ow_d.append(nc.declare_dram_parameter(f"ow{l}", [128, NK * D], BF16, isOutput=False))
        fw1_d.append(nc.declare_dram_parameter(f"fw1{l}", [128, NKF * D], BF16, isOutput=False))
        fw2_d.append(nc.declare_dram_parameter(f"fw2{l}", [128, NK * DFF], BF16, isOutput=False))
    out_d = nc.declare_dram_parameter("out", [128, NK * R], F32, isOutput=True)

    taps = {}
    for tname, tcols in DEBUG_TAPS:
        taps[tname] = nc.declare_dram_parameter(f"tap_{tname}", [128, tcols], BF16, isOutput=True)

    cc_in = [nc.dram_tensor(f"cc_in{l}", [CC_N], BF16) for l in range(LAYERS)]
    cc_out = [nc.dram_tensor(f"cc_out{l}", [2, CC_N], BF16) for l in range(LAYERS)]

    with TileContext(nc) as tc:
        root_cm = tc.tile_pool(name="root", bufs=1)
        root = root_cm.__enter__()
        par = root.tile([128, PAR_COLS], F32)
        nc.sync.dma_start(par[:], par_d[:])
        ones_col = root.tile([128, 1], BF16)
        nc.vector.memset(ones_col[:], 1.0)
        xpos = root.tile([128, NK * R], BF16)
        h = root.tile([128, NK * R], BF16)

        def pc(name, i=0):            # [128,1] param column
            o = PAR_OFF[name] + i
            return par[:, o:o + 1]

        def pr(name):                 # [1,1] scalar at partition 0
            o = PAR_OFF[name]
            return par[0:1, o:o + 1]

        def tap_write(tname, tile_ap):
            if tname in taps:
                nc.sync.dma_start(taps[tname][:, 0:tile_ap.shape[-1]], tile_ap)

        # ---------------- generic helpers ----------------

        def emit_linear(name, wdram, nb, nkt, in_fn, out_sb, bias_name, act, pool_parent_cols=None):
            """out_T[j-block] = sum_kt W_kt^T @ in_kt, ACT-evac with bias+func."""
            with tc.tile_pool(name=f"{name}_w", bufs=3) as wp, \
                 tc.tile_pool(name=f"{name}_p", bufs=2, space="PSUM") as pp:
                K = nkt * 128
                for j in range(nb):
                    wt = wp.tile([128, K], BF16, tag="w", name=f"{name}_wt")
                    nc.sync.dma_start(wt[:], wdram[:, j * K:(j + 1) * K])
                    ps = pp.tile([128, R], F32, tag="p", name=f"{name}_ps")
                    for kt in range(nkt):
                        nc.tensor.matmul(
                            ps[:], wt[:, kt * 128:(kt + 1) * 128], in_fn(kt),
                            start=(kt == 0), stop=(kt == nkt - 1))
                    bias = pc(bias_name, j) if bias_name else 0.0
                    nc.scalar.activation(
                        out_sb[:, j * R:(j + 1) * R], ps[:],
                        act if act is not None else AF.Identity,
                        bias=bias, scale=1.0)

        def emit_ln(name, in_fn, n_tiles, g_name, b_name, eps, out_sb):
            """LayerNorm over features (partition axis across n_tiles k-tiles)."""
            F = n_tiles * 128
            with tc.tile_pool(name=f"{name}_sq", bufs=2) as sqp, \
                 tc.tile_pool(name=f"{name}_st", bufs=1) as stp, \
                 tc.tile_pool(name=f"{name}_bc", bufs=1) as bcp, \
                 tc.tile_pool(name=f"{name}_ps", bufs=1, space="PSUM") as pp:
                sps = pp.tile([1, R], F32, name=f"{name}_sps")
                qps = pp.tile([1, R], F32, name=f"{name}_qps")
                for i in range(n_tiles):
                    t = in_fn(i)
                    nc.tensor.matmul(sps[:], ones_col[:], t,
                                     start=(i == 0), stop=(i == n_tiles - 1),
                                     skip_group_check=True)
                    sq = sqp.tile([128, R], BF16, tag="sq", name=f"{name}_sqt")
                    nc.scalar.activation(sq[:], t, AF.Square)
                    nc.tensor.matmul(qps[:], ones_col[:], sq[:],
                                     start=(i == 0), stop=(i == n_tiles - 1),
                                     skip_group_check=True)
                m_row = stp.tile([1, R], F32, name=f"{name}_m")
                nc.vector.tensor_scalar(m_row[:], sps[:], 1.0 / F, None, OP.mult)
                m2 = stp.tile([1, R], F32, name=f"{name}_m2")
                nc.scalar.activation(m2[:], m_row[:], AF.Square)
                var = stp.tile([1, R], F32, name=f"{name}_v")
                nc.vector.scalar_tensor_tensor(var[:], qps[:], 1.0 / F, m2[:], OP.mult, OP.subtract)
                std = stp.tile([1, R], F32, name=f"{name}_sd")
                nc.scalar.activation(std[:], var[:], AF.Sqrt, bias=pr(eps), scale=1.0)
                rstd = stp.tile([1, R], F32, name=f"{name}_rs")
                nc.vector.reciprocal(rstd[:], std[:])
                rstd_b = stp.tile([1, R], BF16, name=f"{name}_rsb")
                nc.scalar.copy(rstd_b[:], rstd[:])
                nm_b = stp.tile([1, R], BF16, name=f"{name}_nmb")
                nc.vector.scalar_tensor_tensor(nm_b[:], m_row[:], -1.0, rstd[:], OP.mult, OP.mult)
                rstd_bc = bcp.tile([128, R], BF16, name=f"{name}_rbc")
                nm_bc = bcp.tile([128, R], BF16, name=f"{name}_nbc")
                nc.gpsimd.partition_broadcast(rstd_bc[:], rstd_b[:])
                nc.gpsimd.partition_broadcast(nm_bc[:], nm_b[:])
                for i in range(n_tiles):
                    t = in_fn(i)
                    u = sqp.tile([128, R], BF16, tag="sq", name=f"{name}_u")
                    nc.vector.scalar_tensor_tensor(u[:], t, 1.0, rstd_bc[:], OP.mult, OP.mult)
                    u2 = sqp.tile([128, R], BF16, tag="sq", name=f"{name}_u2")
                    nc.vector.tensor_add(u2[:], u[:], nm_bc[:])
                    nc.vector.tensor_scalar(
                        out_sb[:, i * R:(i + 1) * R], u2[:],
                        pc(g_name, i), pc(b_name, i), OP.mult, OP.add)

        # ---------------- prologue ----------------
        with tc.tile_pool(name="prol", bufs=1) as prol:
            x_sb = prol.tile([128, NK * (R + 2)], BF16)
            nc.sync.dma_start(x_sb[:], x_d[:])
            cs_sb = prol.tile([128, 2 * NK * R], BF16)
            nc.sync.dma_start(cs_sb[:], cs_d[:])
            spm = prol.tile([128, 128], BF16)
            nc.sync.dma_start(spm[:], spm_d[:])

            def x512(kt):
                return x_sb[:, kt * (R + 2) + 1: kt * (R + 2) + 1 + R]

            # rotary -> xpos
            with tc.tile_pool(name="rot_ps", bufs=2, space="PSUM") as rpp, \
                 tc.tile_pool(name="rot_t", bufs=2) as rtp:
                for kt in range(NK):
                    swp = rpp.tile([128, R], F32, tag="sw", name="rot_sw")
                    nc.tensor.matmul(swp[:], spm[:], x512(kt), start=True, stop=True)
                    t0 = rtp.tile([128, R], BF16, tag="t0", name="rot_t0")
                    nc.vector.scalar_tensor_tensor(
                        t0[:], x512(kt), 1.0, cs_sb[:, kt * R:(kt + 1) * R], OP.mult, OP.mult)
                    t1 = rtp.tile([128, R], BF16, tag="t1", name="rot_t1")
                    nc.vector.scalar_tensor_tensor(
                        t1[:], swp[:], 1.0, cs_sb[:, (NK + kt) * R:(NK + kt + 1) * R],
                        OP.mult, OP.mult)
                    nc.vector.tensor_add(xpos[:, kt * R:(kt + 1) * R], t0[:], t1[:])

            tap_write("xpos", xpos[:])
            # depthwise conv over time + delta LN
            conv = prol.tile([128, NK * R], BF16)
            for kt in range(NK):
                c0 = prol.tile([128, R], BF16, tag="cv0", name="conv_c0", bufs=2)
                nc.scalar.activation(
                    c0[:], x_sb[:, kt * (R + 2): kt * (R + 2) + R],
                    AF.Identity, bias=pc("cb", kt), scale=pc("cw0", kt))
                c1 = prol.tile([128, R], BF16, tag="cv1", name="conv_c1", bufs=2)
                nc.vector.scalar_tensor_tensor(c1[:], x512(kt), pc("cw1", kt), c0[:], OP.mult, OP.add)
                nc.vector.scalar_tensor_tensor(
                    conv[:, kt * R:(kt + 1) * R],
                    x_sb[:, kt * (R + 2) + 2: kt * (R + 2) + 2 + R],
                    pc("cw2", kt), c1[:], OP.mult, OP.add)
            delta_n = prol.tile([128, NK * R], BF16)
            emit_ln("dln", lambda i: conv[:, i * R:(i + 1) * R], NK, "dlng", "dlnb", "eps5", delta_n)

            tap_write("conv", conv[:])
            tap_write("delta_n", delta_n[:])
            # gate = sigmoid(x @ Wg + bg)
            gate = prol.tile([128, NK * R], BF16)
            emit_linear("dg", dgw_d, NK, NK, x512, gate, "dgb", AF.Sigmoid)

            # delta_x = x + alpha * gate * delta_n
            dx = prol.tile([128, NK * R], BF16)
            for kt in range(NK):
                td = prol.tile([128, R], BF16, tag="dxt", name="dx_t", bufs=2)
                nc.vector.tensor_mul(td[:], gate[:, kt * R:(kt + 1) * R],
                                     delta_n[:, kt * R:(kt + 1) * R])
                nc.vector.scalar_tensor_tensor(
                    dx[:, kt * R:(kt + 1) * R], td[:], pc("alpha"), x512(kt), OP.mult, OP.add)

            tap_write("gate", gate[:])
            tap_write("dx", dx[:])
            # context gate
            gi = prol.tile([128, 2 * NK * R], BF16)

            def cg_in(i):
                src = xpos if i < NK else dx
                return src[:, (i % NK) * R:(i % NK + 1) * R]
            emit_ln("cgln", cg_in, 2 * NK, "cglng", "cglnb", "eps5", gi)
            cgt = prol.tile([128, NK * R], BF16)
            emit_linear("cg1", cgw1_d, NK, 2 * NK,
                        lambda kt: gi[:, kt * R:(kt + 1) * R], cgt, "cgb1", AF.Tanh)
            cgw2_sb = prol.tile([128, NK], BF16)
            nc.sync.dma_start(cgw2_sb[:], cgw2_d[:])
            with tc.tile_pool(name="cg2_ps", bufs=1, space="PSUM") as gpp:
                g_ps = gpp.tile([1, R], F32)
                for kt in range(NK):
                    nc.tensor.matmul(g_ps[:], cgw2_sb[:, kt:kt + 1],
                                     cgt[:, kt * R:(kt + 1) * R],
                                     start=(kt == 0), stop=(kt == NK - 1))
                g_row = prol.tile([1, R], BF16)
                nc.scalar.activation(g_row[:], g_ps[:], AF.Sigmoid, bias=pr("cgb2"), scale=1.0)
            g_bc = prol.tile([128, R], BF16)
            nc.gpsimd.partition_broadcast(g_bc[:], g_row[:])

            # h_pre = 2*xpos + g*(dx - xpos); h = LN(h_pre)*pre_g + pre_b
            pln = prol.tile([128, NK * R], BF16)
            for kt in range(NK):
                dd = prol.tile([128, R], BF16, tag="hc", name="hc_d", bufs=2)
                nc.vector.tensor_sub(dd[:], dx[:, kt * R:(kt + 1) * R],
                                     xpos[:, kt * R:(kt + 1) * R])
                gg = prol.tile([128, R], BF16, tag="hc2", name="hc_g", bufs=2)
                nc.vector.scalar_tensor_tensor(gg[:], dd[:], 1.0, g_bc[:], OP.mult, OP.mult)
                nc.vector.scalar_tensor_tensor(
                    pln[:, kt * R:(kt + 1) * R],
                    xpos[:, kt * R:(kt + 1) * R], 2.0, gg[:], OP.mult, OP.add)
            emit_ln("pre", lambda i: pln[:, i * R:(i + 1) * R], NK, "preg", "preb", "eps6", h)
            tap_write("h0", h[:])

        # ---------------- transformer layers ----------------
        for l in range(LAYERS):
            with tc.tile_pool(name=f"L{l}", bufs=1) as lp:
                h_in = lambda kt: h[:, kt * R:(kt + 1) * R]

                attn = lp.tile([128, NK * R], BF16, name=f"at{l}")
                CW = H * (HD + 1)    # 1040 cols per token chunk
                with tc.tile_pool(name=f"pqk{l}", bufs=1) as pqk:
                    # q,k projection (16 m-blocks: 0-7 q, 8-15 k)
                    qk = pqk.tile([128, 16 * R], BF16, name=f"qk{l}")
                    emit_linear(f"qk{l}", qkw_d[l], 16, NK, h_in, qk, f"qkb{l}", None)
                    tap_write(f"qk{l}", qk[:])

                    with tc.tile_pool(name=f"pv{l}", bufs=1) as pv:
                        # v projection, row-major [tokens, D]
                        v_loc = pv.tile([128, 4 * D], BF16, name=f"vloc{l}")
                        wv_sb = pv.tile([128, NK * D], BF16, name=f"wv{l}")
                        nc.sync.dma_start(wv_sb[:], wv_d[l][:])
                        with tc.tile_pool(name=f"v{l}_ps", bufs=2, space="PSUM") as vpp:
                            for t in range(4):
                                vp = vpp.tile([128, D], F32, tag="vp", name=f"v{l}_ps_t")
                                for kt in range(NK):
                                    for hf in range(2):
                                        nc.tensor.matmul(
                                            vp[:, hf * 512:(hf + 1) * 512],
                                            h[:, kt * R + t * 128: kt * R + (t + 1) * 128],
                                            wv_sb[:, kt * D + hf * 512: kt * D + (hf + 1) * 512],
                                            start=(kt == 0), stop=(kt == NK - 1),
                                            skip_group_check=True)
                                nc.scalar.copy(v_loc[:, t * D:(t + 1) * D], vp[:])

                        # exchange k/v halves within the pair
                        nc.sync.dma_start(
                            cc_in[l][0:CC_K].rearrange("(p c) -> p c", p=128),
                            qk[:, 8 * R:16 * R])
                        nc.sync.dma_start(
                            cc_in[l][CC_K:CC_N].rearrange("(t p m) -> p t m", p=128, m=D),
                            v_loc[:])
                        nc.gpsimd.collective_compute(
                            "AllGather", OP.bypass,
                            replica_groups=[[0, 1], [2, 3], [4, 5], [6, 7]],
                            ins=[cc_in[l][:]], outs=[cc_out[l][:]])

                    with tc.tile_pool(name=f"pkv{l}", bufs=1) as pkv:
                        k_full = pkv.tile([128, NK * L], BF16, name=f"kf{l}")
                        v_aug = pkv.tile([128, NK * CW], BF16, name=f"va{l}")
                        with tc.tile_pool(name=f"pvn{l}", bufs=1) as pvn:
                            v_full = pvn.tile([128, NK * L], BF16, name=f"vf{l}")
                            v_next = pvn.tile([128, NK * D], BF16, name=f"vn{l}")
                            for r in range(2):
                                nc.sync.dma_start(
                                    k_full[:].rearrange("p (kt g rr) -> p kt g rr",
                                                        g=2, rr=R)[:, :, r, :],
                                    cc_out[l][r, 0:CC_K].rearrange("(p kt rr) -> p kt rr",
                                                                   p=128, rr=R))
                                nc.sync.dma_start(
                                    v_full[:].rearrange("p (g t m) -> p g t m",
                                                        g=2, m=D)[:, r, :, :],
                                    cc_out[l][r, CC_K:CC_N].rearrange("(t p m) -> p t m",
                                                                      p=128, m=D))
                            # v_next[tok] = v[(tok+1) mod L] via DMA (crosses partitions)
                            nc.sync.dma_start(v_next[0:127, :], v_full[1:128, :])
                            nc.sync.dma_start(v_next[127:128, 0:(NK - 1) * D],
                                              v_full[0:1, D:NK * D])
                            nc.sync.dma_start(v_next[127:128, (NK - 1) * D:NK * D],
                                              v_full[0:1, 0:D])
                            # v_aug: per token chunk, per head: 64 cols of v' + ones col
                            for t in range(NK):
                                nc.vector.memset(
                                    v_aug[:, t * CW:(t + 1) * CW]
                                    .rearrange("p (h e) -> p h e", e=HD + 1)[:, :, HD:HD + 1], 1.0)
                                dst = v_aug[:, t * CW:(t + 1) * CW] \
                                    .rearrange("p (h e) -> p h e", e=HD + 1)[:, :, 0:HD]
                                cur = v_full[:, t * D:(t + 1) * D] \
                                    .rearrange("p (h e) -> p h e", e=HD)
                                nxt = v_next[:, t * D:(t + 1) * D] \
                                    .rearrange("p (h e) -> p h e", e=HD)
                                nc.vector.tensor_sub(dst, cur, nxt)

                        # attention per head
                        lam_ap = pr(f"lam{l}")
                        with tc.tile_pool(name=f"a{l}_sp", bufs=2, space="PSUM") as spp, \
                             tc.tile_pool(name=f"a{l}_op", bufs=2, space="PSUM") as opp, \
                             tc.tile_pool(name=f"a{l}_e", bufs=3) as ep, \
                             tc.tile_pool(name=f"a{l}_n", bufs=2) as np_:
                            for hh in range(H):
                                lane = (hh % 2) * 64
                                qf = hh // 2
                                q_ap = qk[lane:lane + 64, qf * R:(qf + 1) * R]
                                o_aug = opp.tile([HD + 1, R], F32, tag="oa", name=f"a{l}_oa")
                                for g in range(4):
                                    sp = spp.tile([128, 2 * R], F32, tag="sp", name=f"a{l}_spt")
                                    for j in range(2):
                                        kt = g * 2 + j
                                        k_ap = k_full[lane:lane + 64,
                                                      qf * L + kt * 128: qf * L + (kt + 1) * 128]
                                        nc.tensor.matmul(sp[:, j * R:(j + 1) * R], k_ap, q_ap,
                                                         start=True, stop=True)
                                    e = ep.tile([128, 2 * R], BF16, tag="e", name=f"a{l}_et")
                                    nc.scalar.activation(e[:], sp[:], AF.Exp, scale=SCALE)
                                    for j in range(2):
                                        kt = g * 2 + j
                                        nc.tensor.matmul(
                                            o_aug[:],
                                            v_aug[:, kt * CW + hh * (HD + 1):
                                                  kt * CW + (hh + 1) * (HD + 1)],
                                            e[:, j * R:(j + 1) * R],
                                            start=(kt == 0), stop=(kt == NK - 1),
                                            skip_group_check=True)
                                zr = np_.tile([1, R], F32, tag="zr", name=f"a{l}_zr")
                                nc.vector.reciprocal(zr[:], o_aug[HD:HD + 1, :])
                                zrs = np_.tile([1, R], BF16, tag="zrs", name=f"a{l}_zrs")
                                nc.vector.tensor_scalar(zrs[:], zr[:], lam_ap, None, OP.mult)
                                zbc = np_.tile([128, R], BF16, tag="zbc", name=f"a{l}_zbc")
                                nc.gpsimd.partition_broadcast(zbc[0:64, :], zrs[:], channels=64)
                                acols = (hh // 2) * R
                                if hh % 2 == 0:
                                    nc.vector.tensor_mul(attn[0:64, acols:acols + R],
                                                         o_aug[0:HD, :], zbc[0:64, :])
                                else:
                                    tmp = np_.tile([128, R], BF16, tag="tmp", name=f"a{l}_tm")
                                    nc.vector.tensor_mul(tmp[0:64, :], o_aug[0:HD, :],
                                                         zbc[0:64, :])
                                    nc.sync.dma_start(attn[64:128, acols:acols + R],
                                                      tmp[0:64, :])
                tap_write(f"attn{l}", attn[:])

                # out proj + residual, then n1 LN
                hr = lp.tile([128, NK * R], BF16, name=f"hr{l}")
                with tc.tile_pool(name=f"o{l}_w", bufs=3) as owp, \
                     tc.tile_pool(name=f"o{l}_p", bufs=2, space="PSUM") as opp2:
                    for j in range(NK):
                        wt = owp.tile([128, D], BF16, tag="w", name=f"o{l}_wt")
                        nc.sync.dma_start(wt[:], ow_d[l][:, j * D:(j + 1) * D])
                        ps = opp2.tile([128, R], F32, tag="p", name=f"o{l}_ps")
                        for kt in range(NK):
                            nc.tensor.matmul(ps[:], wt[:, kt * 128:(kt + 1) * 128],
                                             attn[:, kt * R:(kt + 1) * R],
                                             start=(kt == 0), stop=(kt == NK - 1))
                        nc.vector.scalar_tensor_tensor(
                            hr[:, j * R:(j + 1) * R], ps[:], pc(f"outb{l}", j),
                            h[:, j * R:(j + 1) * R], OP.add, OP.add)
                tap_write(f"hr{l}", hr[:])
                h1 = lp.tile([128, NK * R], BF16, name=f"h1{l}")
                emit_ln(f"n1{l}", lambda i: hr[:, i * R:(i + 1) * R], NK,
                        f"n1g{l}", f"n1b{l}", "eps6", h1)

                # FFN
                with tc.tile_pool(name=f"pff{l}", bufs=1) as pff:
                    xf = pff.tile([128, NK * R], BF16, name=f"xf{l}")
                    emit_ln(f"fn{l}", lambda i: h1[:, i * R:(i + 1) * R], NK,
                            f"fng{l}", f"fnb{l}", "eps6", xf)
                    inter = pff.tile([128, NKF * R], BF16, name=f"it{l}")
                    emit_linear(f"f1{l}", fw1_d[l], NKF, NK,
                                lambda kt: xf[:, kt * R:(kt + 1) * R], inter,
                                f"fb1{l}", AF.Gelu_apprx_tanh)
                    l2 = pff.tile([128, NK * R], BF16, name=f"l2{l}")
                    with tc.tile_pool(name=f"f2{l}_w", bufs=2) as fwp, \
                         tc.tile_pool(name=f"f2{l}_p", bufs=2, space="PSUM") as fpp, \
                         tc.tile_pool(name=f"f2{l}_t", bufs=2) as ftp:
                        for j in range(NK):
                            wt = fwp.tile([128, DFF], BF16, tag="w", name=f"f2{l}_wt")
                            nc.sync.dma_start(wt[:], fw2_d[l][:, j * DFF:(j + 1) * DFF])
                            ps = fpp.tile([128, R], F32, tag="p", name=f"f2{l}_ps")
                            for kt in range(NKF):
                                nc.tensor.matmul(ps[:], wt[:, kt * 128:(kt + 1) * 128],
                                                 inter[:, kt * R:(kt + 1) * R],
                                                 start=(kt == 0), stop=(kt == NKF - 1))
                            tb = ftp.tile([128, R], BF16, tag="t", name=f"f2{l}_tb")
                            nc.scalar.activation(tb[:], ps[:], AF.Identity,
                                                 bias=pc(f"fb2{l}", j), scale=1.0)
                            nc.vector.scalar_tensor_tensor(
                                l2[:, j * R:(j + 1) * R], h1[:, j * R:(j + 1) * R], 2.0,
                                tb[:], OP.mult, OP.add)
                    emit_ln(f"n2{l}", lambda i: l2[:, i * R:(i + 1) * R], NK,
                            f"n2g{l}", f"n2b{l}", "eps6", h)
                tap_write(f"h{l+1}", h[:])

        # ---------------- output ----------------
        with tc.tile_pool(name="fin", bufs=1) as finp:
            out_sb = finp.tile([128, NK * R], F32)
            for kt in range(NK):
                nc.vector.scalar_tensor_tensor(
                    out_sb[:, kt * R:(kt + 1) * R], h[:, kt * R:(kt + 1) * R], 0.5,
                    xpos[:, kt * R:(kt + 1) * R], OP.mult, OP.add)
            nc.sync.dma_start(out_d[:], out_sb[:])
        root_cm.__exit__(None, None, None)

    nc.compile()
    return nc


# ---------------------------------------------------------------------------
# host-side sharding / packing

def _fm(a):
    """[F, R'] -> [128, (F/128)*R'] feature-major tile layout (bf16)."""
    F, Rp = a.shape
    return np.ascontiguousarray(
        a.reshape(F // 128, 128, Rp).transpose(1, 0, 2).reshape(128, -1))


def _bf(a):
    return np.asarray(a, dtype=np.float32).astype(ml_dtypes.bfloat16)


def _pack_w(W):
    """[K, M] -> [128, (M/128)*K]: m-block j, cols j*K + kt*128 + mm."""
    K, M = W.shape
    a = W.reshape(K // 128, 128, M // 128, 128)       # kt, p, j, mm
    a = a.transpose(1, 2, 0, 3)                       # p, j, kt, mm
    return np.ascontiguousarray(a.reshape(128, -1))


def _cols(v):
    """[n*128] -> [128, n] param columns."""
    return np.ascontiguousarray(np.asarray(v, np.float32).reshape(-1, 128).T)


def _prep_inputs(x, params):
    p = {k: np.asarray(v, np.float32) for k, v in params.items()}

    par = np.zeros((128, PAR_COLS), np.float32)

    def setp(name, arr):
        o = PAR_OFF[name]
        arr = np.asarray(arr, np.float32)
        par[:, o:o + arr.shape[1]] = arr

    cw = p["delta_conv_w"][:, 0, :]                  # [D, 3]
    setp("cw0", _cols(cw[:, 0]));  setp("cw1", _cols(cw[:, 1]));  setp("cw2", _cols(cw[:, 2]))
    setp("cb", _cols(p["delta_conv_b"]))
    setp("dlng", _cols(p["delta_ln_g"]));  setp("dlnb", _cols(p["delta_ln_b"]))
    setp("dgb", _cols(p["delta_gate_b"]))
    setp("cglng", _cols(p["cg_ln_g"]));  setp("cglnb", _cols(p["cg_ln_b"]))
    setp("cgb1", _cols(p["cg_b1"]))
    setp("preg", _cols(p["pre_g"]));  setp("preb", _cols(p["pre_b"]))
    for l in range(LAYERS):
        setp(f"qkb{l}", _cols(p["qkv_b"][l][:2 * D]))
        setp(f"outb{l}", _cols(p["out_b"][l]))
        setp(f"n1g{l}", _cols(p["n1_g"][l]));  setp(f"n1b{l}", _cols(p["n1_b"][l]))
        setp(f"fng{l}", _cols(p["ffn_ln_g"][l]));  setp(f"fnb{l}", _cols(p["ffn_ln_b"][l]))
        setp(f"fb1{l}", _cols(p["ffn_b1"][l]));  setp(f"fb2{l}", _cols(p["ffn_b2"][l]))
        setp(f"n2g{l}", _cols(p["n2_g"][l]));  setp(f"n2b{l}", _cols(p["n2_b"][l]))
    par[:, PAR_OFF["alpha"]] = float(p["delta_alpha"][0])
    par[:, PAR_OFF["cgb2"]] = float(p["cg_b2"][0])
    par[:, PAR_OFF["lam0"]] = float(p["lam"][0])
    par[:, PAR_OFF["lam1"]] = float(p["lam"][1])
    par[:, PAR_OFF["eps5"]] = 1e-5
    par[:, PAR_OFF["eps6"]] = 1e-6

    # swap permutation (even<->odd partitions)
    spm = np.zeros((128, 128), np.float32)
    idx = np.arange(128)
    spm[idx ^ 1, idx] = 1.0

    shared = {
        "par": par,
        "spm": _bf(spm),
        "cgw2": _bf(np.ascontiguousarray(p["cg_w2"][:, 0].reshape(NK, 128).T)),
        "dgw": _bf(_pack_w(p["delta_gate_w"])),
        "cgw1": _bf(_pack_w(p["cg_w1"])),
    }
    for l in range(LAYERS):
        qkv_w = p["qkv_w"][l]
        shared[f"qkw{l}"] = _bf(_pack_w(qkv_w[:, :2 * D]))
        wv = qkv_w[:, 2 * D:]
        shared[f"wv{l}"] = _bf(np.ascontiguousarray(
            wv.reshape(NK, 128, D).transpose(1, 0, 2).reshape(128, -1)))
        shared[f"ow{l}"] = _bf(_pack_w(p["out_w"][l]))
        shared[f"fw1{l}"] = _bf(_pack_w(p["ffn_w1"][l]))
        shared[f"fw2{l}"] = _bf(_pack_w(p["ffn_w2"][l]))

    # rotary tables (global, sliced per core)
    inv_freq = 1.0 / (10000.0 ** (np.arange(0, D, 2, dtype=np.float64) / D))
    pos = np.arange(L, dtype=np.float64)
    fr = pos[:, None] * inv_freq[None, :]            # [L, D/2]
    cosv, sinv = np.cos(fr), np.sin(fr)
    C2 = np.empty((L, D), np.float32)
    S2 = np.empty((L, D), np.float32)
    C2[:, 0::2] = cosv;  C2[:, 1::2] = cosv
    S2[:, 0::2] = -sinv; S2[:, 1::2] = sinv

    in_maps = []
    for c in range(N_CORES):
        b, half = c // 2, c % 2
        lo = half * R
        xs = np.zeros((D, R + 2), np.float32)        # x^T slab with halo
        xt = np.asarray(x[b], np.float32).T          # [D, L]
        s0, s1 = max(lo - 1, 0), min(lo + R + 1, L)
        xs[:, s0 - (lo - 1):s0 - (lo - 1) + (s1 - s0)] = xt[:, s0:s1]
        cs = np.concatenate([_fm(C2[lo:lo + R].T), _fm(S2[lo:lo + R].T)], axis=1)
        m = dict(shared)
        m["x"] = _bf(_fm(xs))
        m["cs"] = _bf(cs)
        in_maps.append(m)
    return in_maps


_NC_CACHE = {}


def get_nc():
    key = tuple(DEBUG_TAPS)
    if key not in _NC_CACHE:
        _NC_CACHE[key] = _build()
    return _NC_CACHE[key]


def run_cores(x, params):
    nc = get_nc()
    in_maps = _prep_inputs(x, params)
    res = run_bass_kernel_spmd(nc, in_maps, core_ids=list(range(N_CORES)))
    return res


def assemble(results):
    out = np.zeros((B, L, D), np.float32)
    for c in range(N_CORES):
        b, half = c // 2, c % 2
        o = np.asarray(results[c]["out"], np.float32).reshape(128, NK, R)
        out[b, half * R:(half + 1) * R, :] = o.transpose(2, 1, 0).reshape(R, D)
    return out


def kernel(x, mask, params):
    res = run_cores(x, params)
    return assemble([r for r in res.results])
